# revision 1
# baseline (speedup 1.0000x reference)
"""Trainium2 Bass kernel for nn_CMIA_2843268350555 (dual-branch spatial/freq attention).

Strategy: data-parallel over batch (16 samples / 8 cores = 2 per core).
All matmuls in float32r (11-bit mantissa, full PE rate at free-dim>=256).

Per-sample math (C=256 channels, HW=1024):
  vT_b    = (x_b.T @ w_bv.T)            [hw, c]   (b in {spa, frq})
  x       = w_cdc @ [x_spa; x_frq]      [c, hw]   (+b_cdc: no-op through LN)
  xn      = layernorm_rows(x)           [c, hw]   (affine folded into wqkTg)
  xnT     = transpose(xn)               [hw, c]
  qk      = (xnT.T @ wqkTg) = xn@..     [c, 2hw]  -> q [c,hw], k [c,hw]
  kT      = transpose(k)                [hw, c]
  kw_b    = (kT.T @ (scale*w_b.T))      [c, hw]   (associativity: avoids big logits mm)
  logits  = q.T @ kw_b                  [hw(n), hw(j)]
  att_b   = softmax_j(logits + b_b)
  out_b   = x_b + (vT_b.T @ att_b)      [c, hw]
"""
import numpy as np
import ml_dtypes

import concourse.bacc as bacc
import concourse.mybir as mybir
import concourse.tile as tile
from concourse import bass_utils
from concourse.bass import ts, ds
from concourse.masks import make_identity

f32 = mybir.dt.float32
f32r = mybir.dt.float32r
bf16 = mybir.dt.bfloat16

# bf16 for the streamed attention-branch weights (and kT): halves the
# dominant per-sample DMA stream, but costs ~3e-3 scale-rel output error
# vs 4e-4 with f32r. Measured equal-within-noise on HW, so keep f32r.
WS_BF16 = False
WS_DT = bf16 if WS_BF16 else f32r

B, C, H, W = 16, 256, 32, 32
HW = H * W           # 1024
J2 = 2 * HW          # 2048
NCORES = 8
BPC = B // NCORES    # samples per core
CC = C // 128        # 2 channel chunks
NCH = HW // 128      # 8 hw chunks
EPS = 1e-5


def _round_f32r(x: np.ndarray) -> np.ndarray:
    """RNE-round fp32 to fp32r (11 mantissa bits; low 12 bits zero)."""
    x = np.ascontiguousarray(x, dtype=np.float32)
    u = x.view(np.uint32)
    lsb = (u >> np.uint32(12)) & np.uint32(1)
    r = u + np.uint32(0x7FF) + lsb
    return (r & ~np.uint32(0xFFF)).view(np.float32)


_CACHE: dict = {}


def _ws_prep(x: np.ndarray) -> np.ndarray:
    if WS_BF16:
        return np.ascontiguousarray(x, np.float32).astype(ml_dtypes.bfloat16)
    return _round_f32r(x)


def _build(flags, reps=1):
    has_qkb, has_bspa, has_bfrq, has_bsv, has_bfv = flags
    any_bias = any(flags)

    nc = bacc.Bacc("TRN2", target_bir_lowering=False, debug=False,
                   enable_asserts=True, num_devices=NCORES)
    xs_d = nc.dram_tensor("xs", [BPC, C, HW], f32r, kind="ExternalInput").ap()
    xf_d = nc.dram_tensor("xf", [BPC, C, HW], f32r, kind="ExternalInput").ap()
    wcdc_d = nc.dram_tensor("wcdcT", [2 * C, C], f32r, kind="ExternalInput").ap()
    wsv_d = nc.dram_tensor("wsvT", [C, C], f32r, kind="ExternalInput").ap()
    wfv_d = nc.dram_tensor("wfvT", [C, C], f32r, kind="ExternalInput").ap()
    wqk_d = nc.dram_tensor("wqkTg", [HW, J2], f32r, kind="ExternalInput").ap()
    wspa_d = nc.dram_tensor("wspaT", [HW, HW], WS_DT, kind="ExternalInput").ap()
    wfrq_d = nc.dram_tensor("wfrqT", [HW, HW], WS_DT, kind="ExternalInput").ap()
    qkb_d = bspa_d = bfrq_d = bsv_d = bfv_d = None
    if has_qkb:
        qkb_d = nc.dram_tensor("qkb", [1, J2], f32r, kind="ExternalInput").ap()
    if has_bspa:
        bspa_d = nc.dram_tensor("bspa", [1, HW], f32r, kind="ExternalInput").ap()
    if has_bfrq:
        bfrq_d = nc.dram_tensor("bfrq", [1, HW], f32r, kind="ExternalInput").ap()
    if has_bsv:
        bsv_d = nc.dram_tensor("bsv", [1, C], f32r, kind="ExternalInput").ap()
    if has_bfv:
        bfv_d = nc.dram_tensor("bfv", [1, C], f32r, kind="ExternalInput").ap()
    os_d = nc.dram_tensor("os", [BPC, C, HW], f32, kind="ExternalOutput").ap()
    of_d = nc.dram_tensor("of", [BPC, C, HW], f32, kind="ExternalOutput").ap()

    Sqrt = mybir.ActivationFunctionType.Sqrt
    Exp = mybir.ActivationFunctionType.Exp
    SUB = mybir.AluOpType.subtract
    MUL = mybir.AluOpType.mult

    with tile.TileContext(nc) as tc:
        with tc.tile_pool(name="constp", bufs=1) as constp, \
             tc.tile_pool(name="wqkp", bufs=1) as wqkp, \
             tc.tile_pool(name="data", bufs=1) as data, \
             tc.tile_pool(name="xin", bufs=2) as xin, \
             tc.tile_pool(name="wsp", bufs=6) as wsp, \
             tc.tile_pool(name="small", bufs=4) as small, \
             tc.tile_pool(name="attp", bufs=2) as attp, \
             tc.tile_pool(name="resp", bufs=2) as resp:

            # ---- constants / weights (resident) ----
            # DMA queue split: SP(sync) = inputs + ws streams; ACT(scalar) =
            # wqk + output stores; Pool(gpsimd SWDGE) = small constants.
            wcdc_t = constp.tile([128, 4, C], f32r, name="wcdc_t")
            nc.gpsimd.dma_start(out=wcdc_t,
                                in_=wcdc_d.rearrange("(kc p) c -> p kc c", p=128))
            wsv_t = constp.tile([128, CC, C], f32r, name="wsv_t")
            nc.gpsimd.dma_start(out=wsv_t,
                                in_=wsv_d.rearrange("(kc p) c -> p kc c", p=128))
            wfv_t = constp.tile([128, CC, C], f32r, name="wfv_t")
            nc.gpsimd.dma_start(out=wfv_t,
                                in_=wfv_d.rearrange("(kc p) c -> p kc c", p=128))
            ident = constp.tile([128, 128], f32, name="ident")
            make_identity(nc, ident)
            eps_t = constp.tile([128, 1], f32, name="eps_t")
            nc.vector.memset(eps_t, EPS)
            ones_t = None
            if any_bias:
                ones_f = constp.tile([1, 128], f32, name="ones_f")
                nc.vector.memset(ones_f, 1.0)
                ones_t = constp.tile([1, 128], f32r, name="ones_t")
                nc.scalar.copy(out=ones_t, in_=ones_f)

            def _bias_tile(dram, n, nm):
                t = constp.tile([1, n], f32r, name=nm)
                nc.gpsimd.dma_start(out=t, in_=dram)
                return t

            qkb_t = _bias_tile(qkb_d, J2, "qkb_t") if has_qkb else None
            bspa_t = _bias_tile(bspa_d, HW, "bspa_t") if has_bspa else None
            bfrq_t = _bias_tile(bfrq_d, HW, "bfrq_t") if has_bfrq else None
            bsv_t = _bias_tile(bsv_d, C, "bsv_t") if has_bsv else None
            bfv_t = _bias_tile(bfv_d, C, "bfv_t") if has_bfv else None

            # wqk split across the ACT HWDGE queue and the Pool SWDGE queue
            # (SP stays free for inputs/ws) so stage D's K-chunks land early.
            wqk_t = wqkp.tile([128, NCH, J2], f32r, name="wqk_t")
            for kc in range(NCH):
                eng = nc.scalar if kc < 4 else nc.gpsimd
                eng.dma_start(
                    out=wqk_t[:, kc, :],
                    in_=wqk_d[ds(kc * 128, 128), :])

            def _samples_body():
              for b in range(BPC):
                xs_t = xin.tile([128, CC, HW], f32r, tag="xs", name=f"xs{b}")
                nc.sync.dma_start(
                    out=xs_t, in_=xs_d[b].rearrange("(cc p) n -> p cc n", p=128))
                xf_t = xin.tile([128, CC, HW], f32r, tag="xf", name=f"xf{b}")
                nc.sync.dma_start(
                    out=xf_t, in_=xf_d[b].rearrange("(cc p) n -> p cc n", p=128))

                vts = data.tile([128, NCH, C], f32r, tag="vts", name=f"vts{b}")
                vtf = data.tile([128, NCH, C], f32r, tag="vtf", name=f"vtf{b}")
                x_sb = data.tile([128, CC, HW], f32, tag="xc", name=f"x_sb{b}")
                xnT = data.tile([128, NCH, C], f32r, tag="tp", name=f"xnT{b}")

                # One shared matmul-psum pool (3x512) + transpose pool (2)
                # across stages A-D avoids per-stage PSUM zone churn.
                with tc.tile_pool(name="psMM", bufs=4, space="PSUM") as psMM, \
                     tc.tile_pool(name="psT", bufs=2, space="PSUM") as psT:
                    # ---- A: value projections, transposed ----
                    for src, wv, dst, bt in ((xs_t, wsv_t, vts, bsv_t),
                                             (xf_t, wfv_t, vtf, bfv_t)):
                        for mc in range(NCH):
                            ps = psMM.tile([128, 512], f32, tag="ps", name="psa")
                            for kc in range(CC):
                                nc.tensor.matmul(
                                    ps[:, 0:C],
                                    src[:, kc, ts(mc, 128)], wv[:, kc, :],
                                    start=(kc == 0),
                                    stop=(kc == CC - 1 and bt is None))
                            if bt is not None:
                                nc.tensor.matmul(ps[:, 0:C], ones_t, bt,
                                                 start=False, stop=True)
                            nc.vector.tensor_copy(out=dst[:, mc, :],
                                                  in_=ps[:, 0:C])

                    # ---- B: x = w_cdc @ [xs; xf] ----
                    for cc in range(CC):
                        for nn in range(2):
                            ps = psMM.tile([128, 512], f32, tag="ps", name="psb")
                            for kc in range(4):
                                src = xs_t if kc < 2 else xf_t
                                nc.tensor.matmul(
                                    ps, wcdc_t[:, kc, ts(cc, 128)],
                                    src[:, kc % 2, ds(nn * 512, 512)],
                                    start=(kc == 0), stop=(kc == 3))
                            nc.scalar.copy(out=x_sb[:, cc, ds(nn * 512, 512)],
                                           in_=ps)

                    # ---- LayerNorm rows of x (in place) ----
                    for cc in range(CC):
                        xr = x_sb[:, cc, :].rearrange("p (s f) -> p s f", s=2)
                        stats = small.tile([128, 2, 6], f32, tag="st",
                                           name="stats")
                        for s in range(2):
                            nc.vector.bn_stats(out=stats[:, s, :],
                                               in_=xr[:, s, :])
                        mv = small.tile([128, 2], f32, tag="mv", name="mv")
                        nc.vector.bn_aggr(out=mv, in_=stats)
                        rstd = small.tile([128, 1], f32, tag="rstd", name="rstd")
                        nc.scalar.activation(out=rstd, in_=mv[:, 1:2], func=Sqrt,
                                             bias=eps_t, scale=1.0)
                        nc.vector.reciprocal(out=rstd, in_=rstd)
                        nc.vector.tensor_scalar(
                            out=x_sb[:, cc, :], in0=x_sb[:, cc, :],
                            scalar1=mv[:, 0:1], scalar2=rstd, op0=SUB, op1=MUL)

                    # ---- C: xnT = xn.T ----  (xnT shares slot with kT)
                    for cc in range(CC):
                        for dc in range(NCH):
                            pt = psT.tile([128, 128], f32, tag="pt", name="pt")
                            nc.tensor.transpose(
                                pt, x_sb[:, cc, ds(dc * 128, 128)], ident)
                            nc.scalar.copy(out=xnT[:, dc, ts(cc, 128)], in_=pt)

                    # ---- D: qk = xn @ wqkTg ----
                    q_t = data.tile([128, CC, HW], f32r, tag="q", name=f"q{b}")
                    k_sb = data.tile([128, CC, HW], f32, tag="xc",
                                     name=f"k_sb{b}")
                    # dc middle / nn inner: the stationary xnT chunk is
                    # reused across 4 consecutive matmuls (LDWEIGHTS amortize)
                    for cc in range(CC):
                        psd = [psMM.tile([128, 512], f32, tag="ps", bufs=4,
                                         name=f"psd{b}_{cc}_{nn}")
                               for nn in range(4)]
                        for dc in range(NCH):
                            for nn in range(4):
                                nc.tensor.matmul(
                                    psd[nn], xnT[:, dc, ts(cc, 128)],
                                    wqk_t[:, dc, ds(nn * 512, 512)],
                                    start=(dc == 0),
                                    stop=(dc == NCH - 1 and not has_qkb))
                        for nn in range(4):
                            if has_qkb:
                                nc.tensor.matmul(
                                    psd[nn], ones_t, qkb_t[:, ds(nn * 512, 512)],
                                    start=False, stop=True)
                            if nn < 2:
                                nc.scalar.copy(
                                    out=q_t[:, cc, ds(nn * 512, 512)],
                                    in_=psd[nn])
                            else:
                                nc.vector.tensor_copy(
                                    out=k_sb[:, cc, ds((nn - 2) * 512, 512)],
                                    in_=psd[nn])

                    # ---- kT = k.T ----
                    kT = data.tile([128, NCH, C], WS_DT, tag="tp", name=f"kT{b}")
                    for cc in range(CC):
                        for mc in range(NCH):
                            pt = psT.tile([128, 128], f32, tag="pt", name="pt2")
                            nc.tensor.transpose(
                                pt, k_sb[:, cc, ds(mc * 128, 128)], ident)
                            nc.scalar.copy(out=kT[:, mc, ts(cc, 128)], in_=pt)

                # ---- branches ----
                # One PSUM pool for E/F/G of both branches: tag "pl" (2 slots)
                # serves both E's accumulators and F's logits tiles, so the
                # next branch's E starts as soon as a logits slot frees
                # (no pool-close barrier on the residual reads).
                with tc.tile_pool(name="psBR", bufs=1, space="PSUM") as psBR:
                  for br, (wsd, lb_t, vt, out_d, x_res) in enumerate((
                        (wspa_d, bspa_t, vts, os_d, xs_t),
                        (wfrq_d, bfrq_t, vtf, of_d, xf_t))):
                    # E: kw = k @ (scale * w.T); ws chunk DMAs issued up
                    # front (bufs=4 pool) so they prefetch during earlier
                    # stages on the SP queue.
                    kw = data.tile([128, CC, HW], f32r, tag=f"kw{br}",
                                   name=f"kw{b}_{br}")
                    ws_tiles = []
                    for mc in range(NCH):
                        wst = wsp.tile([128, HW], WS_DT, tag="ws",
                                       name=f"ws{b}_{br}_{mc}")
                        nc.sync.dma_start(out=wst,
                                          in_=wsd[ds(mc * 128, 128), :])
                        ws_tiles.append(wst)
                    pse = [psBR.tile([128, HW], f32, tag="pl", bufs=2,
                                     name=f"pse{b}_{br}_{cc}")
                           for cc in range(CC)]
                    for mc in range(NCH):
                        for cc in range(CC):
                            for jj in range(2):
                                nc.tensor.matmul(
                                    pse[cc][:, ds(jj * 512, 512)],
                                    kT[:, mc, ts(cc, 128)],
                                    ws_tiles[mc][:, ds(jj * 512, 512)],
                                    start=(mc == 0), stop=(mc == NCH - 1))
                    for cc in range(CC):
                        for jj in range(2):
                            cp = (nc.vector.tensor_copy if jj == 0
                                  else nc.scalar.copy)
                            cp(out=kw[:, cc, ds(jj * 512, 512)],
                               in_=pse[cc][:, ds(jj * 512, 512)])

                    # F/G: logits -> exp(+rowsum) -> out accumulation.
                    # 1/rowsum folds into the small vT chunk, not the big att.
                    if True:
                        psg = psBR.tile([128, CC, HW], f32, tag="psg", bufs=1,
                                        name=f"psg{b}_{br}")
                        for nk in range(NCH):
                            pl = psBR.tile([128, HW], f32, tag="pl", bufs=2,
                                           name="pl")
                            for cc in range(CC):
                                for jj in range(2):
                                    nc.tensor.matmul(
                                        pl[:, ds(jj * 512, 512)],
                                        q_t[:, cc, ts(nk, 128)],
                                        kw[:, cc, ds(jj * 512, 512)],
                                        start=(cc == 0),
                                        stop=(cc == CC - 1 and lb_t is None))
                            if lb_t is not None:
                                for jj in range(2):
                                    nc.tensor.matmul(
                                        pl[:, ds(jj * 512, 512)], ones_t,
                                        lb_t[:, ds(jj * 512, 512)],
                                        start=False, stop=True)
                            et = attp.tile([128, HW], f32r, tag="att",
                                           name=f"et{b}_{br}_{nk}")
                            rsum = small.tile([128, 1], f32, tag="rs", name="rsum")
                            nc.scalar.activation(out=et, in_=pl, func=Exp,
                                                 accum_out=rsum)
                            rrec = small.tile([128, 1], f32, tag="rr", name="rrec")
                            nc.vector.reciprocal(out=rrec, in_=rsum)
                            vtn = small.tile([128, C], f32r, tag="vtn",
                                             name="vtn")
                            nc.vector.tensor_scalar_mul(out=vtn,
                                                        in0=vt[:, nk, :],
                                                        scalar1=rrec)
                            for cc in range(CC):
                                for jj in range(2):
                                    nc.tensor.matmul(
                                        psg[:, cc, ds(jj * 512, 512)],
                                        vtn[:, ts(cc, 128)],
                                        et[:, ds(jj * 512, 512)],
                                        start=(nk == 0), stop=(nk == NCH - 1))
                        for cc in range(CC):
                            res = resp.tile([128, HW], f32, tag="res",
                                            name=f"res{b}_{br}_{cc}", bufs=1)
                            nc.vector.tensor_add(out=res, in0=psg[:, cc, :],
                                                 in1=x_res[:, cc, :].bitcast(f32))
                            nc.scalar.dma_start(
                                out=out_d[b, ds(cc * 128, 128), :], in_=res)

            if reps == 1:
                _samples_body()
            elif isinstance(reps, tuple):      # ("unroll", R)
                for _rep in range(reps[1]):
                    _samples_body()
            else:
                with tc.For_i(0, reps, 1):
                    _samples_body()

    nc.compile()
    return nc


def kernel(x_spa, x_freq, w_cdc, b_cdc, w_sv, b_sv, w_fv, b_fv,
           ln_w, ln_b, w_qk, w_spa, b_spa, w_frq, b_frq):
    x_spa = np.asarray(x_spa, np.float32)
    x_freq = np.asarray(x_freq, np.float32)
    w_cdc = np.asarray(w_cdc, np.float32)
    w_sv = np.asarray(w_sv, np.float32)
    w_fv = np.asarray(w_fv, np.float32)
    ln_w = np.asarray(ln_w, np.float32)
    ln_b = np.asarray(ln_b, np.float32)
    w_qk = np.asarray(w_qk, np.float32)
    w_spa = np.asarray(w_spa, np.float32)
    w_frq = np.asarray(w_frq, np.float32)
    b_sv = np.asarray(b_sv, np.float32)
    b_fv = np.asarray(b_fv, np.float32)
    b_spa = np.asarray(b_spa, np.float32)
    b_frq = np.asarray(b_frq, np.float32)
    # b_cdc is a per-row constant added before LayerNorm over that row: no-op.

    scale = float(HW) ** -0.5
    qkb = ln_b @ w_qk.T                      # [2hw]
    flags = (bool(np.any(qkb)), bool(np.any(b_spa)), bool(np.any(b_frq)),
             bool(np.any(b_sv)), bool(np.any(b_fv)))

    if flags not in _CACHE:
        _CACHE[flags] = _build(flags)
    nc = _CACHE[flags]

    xs = _round_f32r(x_spa.reshape(B, C, HW))
    xf = _round_f32r(x_freq.reshape(B, C, HW))
    base = {
        "wcdcT": _round_f32r(w_cdc.T),
        "wsvT": _round_f32r(w_sv.T),
        "wfvT": _round_f32r(w_fv.T),
        "wqkTg": _round_f32r(w_qk.T * ln_w[:, None]),
        "wspaT": _ws_prep(w_spa.T * scale),
        "wfrqT": _ws_prep(w_frq.T * scale),
    }
    if flags[0]:
        base["qkb"] = _round_f32r(qkb[None, :])
    if flags[1]:
        base["bspa"] = _round_f32r(b_spa[None, :])
    if flags[2]:
        base["bfrq"] = _round_f32r(b_frq[None, :])
    if flags[3]:
        base["bsv"] = _round_f32r(b_sv[None, :])
    if flags[4]:
        base["bfv"] = _round_f32r(b_fv[None, :])

    in_maps = []
    for c in range(NCORES):
        m = dict(base)
        m["xs"] = xs[c * BPC:(c + 1) * BPC]
        m["xf"] = xf[c * BPC:(c + 1) * BPC]
        in_maps.append(m)

    res = bass_utils.run_bass_kernel_spmd(nc, in_maps, core_ids=list(range(NCORES)))
    out_spa = np.concatenate([res.results[c]["os"] for c in range(NCORES)], axis=0)
    out_frq = np.concatenate([res.results[c]["of"] for c in range(NCORES)], axis=0)
    return (out_spa.reshape(B, C, H, W).astype(np.float32),
            out_frq.reshape(B, C, H, W).astype(np.float32))



# revision 16
# speedup vs baseline: 1.5751x; 1.5751x over previous
"""Trainium2 Bass kernel for nn_CMIA_2843268350555 (dual-branch spatial/freq attention).

Strategy: data-parallel over batch (16 samples / 8 cores = 2 per core).

Host-side algebra: fold w_k into the branch weights,
    W_b = (w_qk.T[:, hw:] * ln_w) @ (w_b.T * hw**-0.5)      (b in {spa, frq})
so the device never computes k, never transposes k, and never streams
w_spa/w_frq: one resident [hw, 3hw] bf16 weight serves q and both kw branches.

Per-sample device math (C=256, HW=1024):
  vT_b  = (x_b.T @ w_bv.T)                  [hw, c]   bf16  (A)
  x     = w_cdc @ [x_spa; x_frq]            [c, hw]   bf16  (B)
  xn    = layernorm_rows(x) in place        (ln_w folded into wbig, ln_b -> bias rows)
  xnT   = xn.T  (PE transposes)             [hw, c]   bf16  (C)
  [q | kw_spa | kw_frq] = xn @ wbig         [c, 3hw]        (D)
  logits_b = q.T @ kw_b                     [hw, hw]        (F)
  att_b = exp(logits_b)  (+rowsum; 1/rowsum folds into small vT)
  out_b = x_b + (vT_b/rowsum).T @ att_b     [c, hw]         (G)

Emission is stage-major across the two samples (AB0 AB1 LNCD0 LNCD1 FG0 FG1)
so DVE/ACT chains of one sample hide under PE work of the other.
PSUM: tag "pl" [128,1024] bufs=2 (4 banks) + tag "psg" [128,2,1024] bufs=1
(4 banks) = all 8 banks, no pool churn.
"""
import numpy as np
import ml_dtypes

import concourse.bacc as bacc
import concourse.mybir as mybir
import concourse.tile as tile
from concourse import bass_utils
from concourse.bass import ts, ds
from concourse.masks import make_identity

f32 = mybir.dt.float32
f32r = mybir.dt.float32r
bf16 = mybir.dt.bfloat16

B, C, H, W = 16, 256, 32, 32
HW = H * W           # 1024
NCORES = 8
BPC = B // NCORES    # samples per core
CC = C // 128        # 2 channel chunks
NCH = HW // 128      # 8 hw chunks
EPS = 1e-5


def _round_f32r(x: np.ndarray) -> np.ndarray:
    """RNE-round fp32 to fp32r (11 mantissa bits; low 12 bits zero)."""
    x = np.ascontiguousarray(x, dtype=np.float32)
    u = x.view(np.uint32)
    lsb = (u >> np.uint32(12)) & np.uint32(1)
    r = u + np.uint32(0x7FF) + lsb
    return (r & ~np.uint32(0xFFF)).view(np.float32)


def _bf(x: np.ndarray) -> np.ndarray:
    return np.ascontiguousarray(x, np.float32).astype(ml_dtypes.bfloat16)


_CACHE: dict = {}


def _build(flags, reps=1, dbg=False):
    has_dqb, has_bspa, has_bfrq, has_bsv, has_bfv = flags

    nc = bacc.Bacc("TRN2", target_bir_lowering=False, debug=False,
                   enable_asserts=True, num_devices=NCORES)
    dbg_d = {}
    if dbg:
        for nm, shp, dt in (("d_vts", [128, NCH, C], bf16),
                            ("d_xsb", [128, CC, HW], bf16),
                            ("d_xnT", [128, NCH, C], bf16),
                            ("d_q", [128, CC, HW], bf16),
                            ("d_kw", [128, CC, 2 * HW], bf16),
                            ("d_et", [128, HW], bf16)):
            dbg_d[nm] = nc.dram_tensor(nm, shp, dt, kind="ExternalOutput").ap()
    xs_d = nc.dram_tensor("xs", [BPC, C, HW], f32r, kind="ExternalInput").ap()
    xf_d = nc.dram_tensor("xf", [BPC, C, HW], f32r, kind="ExternalInput").ap()
    wcdc_d = nc.dram_tensor("wcdcT", [2 * C, C], f32r, kind="ExternalInput").ap()
    wsv_d = nc.dram_tensor("wsvT", [C, C], f32r, kind="ExternalInput").ap()
    wfv_d = nc.dram_tensor("wfvT", [C, C], f32r, kind="ExternalInput").ap()
    wbig_d = nc.dram_tensor("wbig", [HW, 3 * HW], bf16, kind="ExternalInput").ap()
    dqb_d = bspa_d = bfrq_d = bsv_d = bfv_d = None
    if has_dqb:
        dqb_d = nc.dram_tensor("dqb", [1, 3 * HW], bf16, kind="ExternalInput").ap()
    if has_bspa:
        bspa_d = nc.dram_tensor("bspa", [1, HW], bf16, kind="ExternalInput").ap()
    if has_bfrq:
        bfrq_d = nc.dram_tensor("bfrq", [1, HW], bf16, kind="ExternalInput").ap()
    if has_bsv:
        bsv_d = nc.dram_tensor("bsv", [1, C], f32r, kind="ExternalInput").ap()
    if has_bfv:
        bfv_d = nc.dram_tensor("bfv", [1, C], f32r, kind="ExternalInput").ap()
    os_d = nc.dram_tensor("os", [BPC, C, HW], f32, kind="ExternalOutput").ap()
    of_d = nc.dram_tensor("of", [BPC, C, HW], f32, kind="ExternalOutput").ap()

    Sqrt = mybir.ActivationFunctionType.Sqrt
    Exp = mybir.ActivationFunctionType.Exp
    SUB = mybir.AluOpType.subtract
    MUL = mybir.AluOpType.mult

    with tile.TileContext(nc) as tc:
        with tc.tile_pool(name="constp", bufs=1) as constp, \
             tc.tile_pool(name="wbigp", bufs=1) as wbigp, \
             tc.tile_pool(name="xin", bufs=2) as xin, \
             tc.tile_pool(name="data", bufs=2) as data, \
             tc.tile_pool(name="small", bufs=4) as small, \
             tc.tile_pool(name="attp", bufs=3) as attp, \
             tc.tile_pool(name="vtnp", bufs=4) as vtnp, \
             tc.tile_pool(name="resp", bufs=2) as resp, \
             tc.tile_pool(name="psPL", bufs=2, space="PSUM") as psPL, \
             tc.tile_pool(name="psG", bufs=1, space="PSUM") as psG:

            # ---- resident constants / weights ----
            wcdc_t = constp.tile([128, 4, C], f32r, name="wcdc_t")
            nc.sync.dma_start(out=wcdc_t,
                              in_=wcdc_d.rearrange("(kc p) c -> p kc c", p=128))
            wsv_t = constp.tile([128, CC, C], f32r, name="wsv_t")
            nc.sync.dma_start(out=wsv_t,
                              in_=wsv_d.rearrange("(kc p) c -> p kc c", p=128))
            wfv_t = constp.tile([128, CC, C], f32r, name="wfv_t")
            nc.sync.dma_start(out=wfv_t,
                              in_=wfv_d.rearrange("(kc p) c -> p kc c", p=128))
            ident = constp.tile([128, 128], bf16, name="ident")
            make_identity(nc, ident)
            eps_t = constp.tile([128, 1], f32, name="eps_t")
            nc.vector.memset(eps_t, EPS)
            ones_bf = ones_fr = None
            if has_dqb or has_bspa or has_bfrq:
                ones_bf = constp.tile([1, 128], bf16, name="ones_bf")
                nc.vector.memset(ones_bf, 1.0)
            if has_bsv or has_bfv:
                ones_fr = constp.tile([1, 128], f32, name="ones_f")
                nc.vector.memset(ones_fr, 1.0)
                ones_fr2 = constp.tile([1, 128], f32r, name="ones_fr")
                nc.scalar.copy(out=ones_fr2, in_=ones_fr)
                ones_fr = ones_fr2

            def _bias_tile(dram, n, nm, dt):
                t = constp.tile([1, n], dt, name=nm)
                nc.sync.dma_start(out=t, in_=dram)
                return t

            dqb_t = _bias_tile(dqb_d, 3 * HW, "dqb_t", bf16) if has_dqb else None
            bspa_t = _bias_tile(bspa_d, HW, "bspa_t", bf16) if has_bspa else None
            bfrq_t = _bias_tile(bfrq_d, HW, "bfrq_t", bf16) if has_bfrq else None
            bsv_t = _bias_tile(bsv_d, C, "bsv_t", f32r) if has_bsv else None
            bfv_t = _bias_tile(bfv_d, C, "bfv_t", f32r) if has_bfv else None

            wbig_t = wbigp.tile([128, NCH, 3 * HW], bf16, name="wbig_t")
            for dc in range(NCH):
                nc.scalar.dma_start(out=wbig_t[:, dc, :],
                                    in_=wbig_d[ds(dc * 128, 128), :])

            def _stage_AB(b, xs_t, xf_t):
                nc.sync.dma_start(
                    out=xs_t, in_=xs_d[b].rearrange("(cc p) n -> p cc n", p=128))
                nc.gpsimd.dma_start(
                    out=xf_t, in_=xf_d[b].rearrange("(cc p) n -> p cc n", p=128))

                vts = data.tile([128, NCH, C], bf16, tag="vts", name=f"vts{b}")
                vtf = data.tile([128, NCH, C], bf16, tag="vtf", name=f"vtf{b}")
                x_sb = data.tile([128, CC, HW], bf16, tag="xc", name=f"x_sb{b}")

                # A: vT = x.T @ wv.T, 4 mc chunks (4x256) per [128,1024] pl tile
                for src, wv, dst, bt in ((xs_t, wsv_t, vts, bsv_t),
                                         (xf_t, wfv_t, vtf, bfv_t)):
                    for mq in range(2):
                        pl = psPL.tile([128, 1024], f32, tag="pl", name="plA")
                        for kc in range(CC):
                            for j in range(4):
                                mc = 4 * mq + j
                                nc.tensor.matmul(
                                    pl[:, ds(j * 256, 256)],
                                    src[:, kc, ts(mc, 128)], wv[:, kc, :],
                                    start=(kc == 0 and j % 2 == 0),
                                    stop=(kc == CC - 1 and bt is None),
                                    skip_group_check=True)
                        if bt is not None:
                            for j in range(4):
                                nc.tensor.matmul(pl[:, ds(j * 256, 256)],
                                                 ones_fr, bt, start=False,
                                                 stop=True, skip_group_check=True)
                        nc.vector.tensor_copy(
                            out=dst[:, ds(4 * mq, 4), :],
                            in_=pl.rearrange("p (j c) -> p j c", j=4))

                # B: x = w_cdc @ [xs; xf]
                for cc in range(CC):
                    pl = psPL.tile([128, 1024], f32, tag="pl", name="plB")
                    for kc in range(4):
                        src = xs_t if kc < 2 else xf_t
                        for nn in range(2):
                            nc.tensor.matmul(
                                pl[:, ds(nn * 512, 512)],
                                wcdc_t[:, kc, ts(cc, 128)],
                                src[:, kc % 2, ds(nn * 512, 512)],
                                start=(kc == 0), stop=(kc == 3))
                    nc.vector.tensor_copy(out=x_sb[:, cc, :], in_=pl)
                return vts, vtf, x_sb

            def _stage_LN(b, x_sb):
                # LayerNorm rows of x, in place (bf16)
                for cc in range(CC):
                    xr = x_sb[:, cc, :].rearrange("p (s f) -> p s f", s=2)
                    stats = small.tile([128, 2, 6], f32, tag="st", name="stats")
                    for s in range(2):
                        nc.vector.bn_stats(out=stats[:, s, :], in_=xr[:, s, :])
                    mv = small.tile([128, 2], f32, tag="mv", name="mv")
                    nc.vector.bn_aggr(out=mv, in_=stats)
                    rstd = small.tile([128, 1], f32, tag="rstd", name="rstd")
                    nc.scalar.activation(out=rstd, in_=mv[:, 1:2], func=Sqrt,
                                         bias=eps_t, scale=1.0)
                    nc.vector.reciprocal(out=rstd, in_=rstd)
                    nc.vector.tensor_scalar(
                        out=x_sb[:, cc, :], in0=x_sb[:, cc, :],
                        scalar1=mv[:, 0:1], scalar2=rstd, op0=SUB, op1=MUL)

            def _stage_CD(b, x_sb):
                # C: xnT = xn.T  (8 transposes packed per [128,1024] pl tile)
                xnT = data.tile([128, NCH, C], bf16, tag="tp", name=f"xnT{b}")
                for cc in range(CC):
                    pl = psPL.tile([128, 1024], bf16, tag="pl", name="plC")
                    for dc in range(NCH):
                        nc.tensor.matmul(
                            pl[:, ds(dc * 128, 128)],
                            x_sb[:, cc, ds(dc * 128, 128)], ident,
                            is_transpose=True,
                            start=(dc == 0), stop=(dc == NCH - 1),
                            skip_group_check=True)
                    nc.scalar.copy(
                        out=xnT[:, :, ts(cc, 128)],
                        in_=pl.rearrange("p (dc c) -> p dc c", dc=NCH))

                # D: [q | kw_spa | kw_frq] = xn @ wbig
                q_t = data.tile([128, CC, HW], bf16, tag="q", name=f"q{b}")
                kw_t = data.tile([128, CC, 2 * HW], bf16, tag="kw", name=f"kw{b}")
                for cc in range(CC):
                    for np_ in range(3):
                        pl = psPL.tile([128, 1024], f32, tag="pl", name="plD")
                        for dc in range(NCH):
                            for h in range(2):
                                nn = np_ * 2 + h
                                nc.tensor.matmul(
                                    pl[:, ds(h * 512, 512)],
                                    xnT[:, dc, ts(cc, 128)],
                                    wbig_t[:, dc, ds(nn * 512, 512)],
                                    start=(dc == 0),
                                    stop=(dc == NCH - 1 and not has_dqb))
                        if has_dqb:
                            for h in range(2):
                                nc.tensor.matmul(
                                    pl[:, ds(h * 512, 512)], ones_bf,
                                    dqb_t[:, ds((np_ * 2 + h) * 512, 512)],
                                    start=False, stop=True)
                        if np_ == 0:
                            nc.vector.tensor_copy(out=q_t[:, cc, :], in_=pl)
                        elif np_ == 1:
                            nc.scalar.copy(out=kw_t[:, cc, 0:HW], in_=pl)
                        else:
                            nc.vector.tensor_copy(out=kw_t[:, cc, HW:2 * HW],
                                                  in_=pl)
                return xnT, q_t, kw_t

            def _stage_FG(b, xs_t, xf_t, vts, vtf, q_t, kw_t, dbg_o=None):
                for br, (lb_t, vt, out_d, x_res) in enumerate(
                        ((bspa_t, vts, os_d, xs_t),
                         (bfrq_t, vtf, of_d, xf_t))):
                    psg = psG.tile([128, CC, HW], f32, tag="psg",
                                   name=f"psg{b}_{br}")
                    ets, vtns = [None] * NCH, [None] * NCH

                    def _F(nk):
                        pl = psPL.tile([128, 1024], f32, tag="pl", name="plF")
                        for cc in range(CC):
                            for jj in range(2):
                                nc.tensor.matmul(
                                    pl[:, ds(jj * 512, 512)],
                                    q_t[:, cc, ts(nk, 128)],
                                    kw_t[:, cc, ds(br * HW + jj * 512, 512)],
                                    start=(cc == 0),
                                    stop=(cc == CC - 1 and lb_t is None))
                        if lb_t is not None:
                            for jj in range(2):
                                nc.tensor.matmul(
                                    pl[:, ds(jj * 512, 512)], ones_bf,
                                    lb_t[:, ds(jj * 512, 512)],
                                    start=False, stop=True)
                        et = attp.tile([128, HW], bf16, tag="att",
                                       name=f"et{b}_{br}_{nk}")
                        rsum = small.tile([128, 1], f32, tag="rs", name="rsum")
                        nc.scalar.activation(out=et, in_=pl, func=Exp,
                                             accum_out=rsum)
                        rrec = small.tile([128, 1], f32, tag="rr", name="rrec")
                        nc.vector.reciprocal(out=rrec, in_=rsum)
                        vtn = vtnp.tile([128, C], bf16, tag="vtn", name="vtn")
                        nc.vector.tensor_scalar_mul(out=vtn, in0=vt[:, nk, :],
                                                    scalar1=rrec)
                        ets[nk], vtns[nk] = et, vtn

                    def _G(nk):
                        for cc in range(CC):
                            for jj in range(2):
                                nc.tensor.matmul(
                                    psg[:, cc, ds(jj * 512, 512)],
                                    vtns[nk][:, ts(cc, 128)],
                                    ets[nk][:, ds(jj * 512, 512)],
                                    start=(nk == 0), stop=(nk == NCH - 1))

                    # software-pipeline F ahead of G by one nk slot
                    _F(0)
                    for nk in range(1, NCH):
                        _F(nk)
                        _G(nk - 1)
                    _G(NCH - 1)
                    if dbg_o is not None and br == 0:
                        nc.gpsimd.dma_start(out=dbg_o["d_et"], in_=ets[0])

                    res = resp.tile([128, CC, HW], f32, tag="res",
                                    name=f"res{b}_{br}")
                    for cc in range(CC):
                        nc.vector.tensor_add(out=res[:, cc, :],
                                             in0=psg[:, cc, :],
                                             in1=x_res[:, cc, :].bitcast(f32))
                        eng = nc.gpsimd if (br + cc) % 2 == 0 else nc.sync
                        eng.dma_start(out=out_d[b, ds(cc * 128, 128), :],
                                      in_=res[:, cc, :])

            def _samples_body(mark=None):
                xs_ts, xf_ts, st = [], [], []
                for b in range(BPC):
                    xs_t = xin.tile([128, CC, HW], f32r, tag="xs", name=f"xs{b}")
                    xf_t = xin.tile([128, CC, HW], f32r, tag="xf", name=f"xf{b}")
                    xs_ts.append(xs_t)
                    xf_ts.append(xf_t)
                    st.append(_stage_AB(b, xs_t, xf_t))
                    _stage_LN(b, st[b][2])
                if mark:
                    mark()
                dt = []
                for b in range(BPC):
                    dt.append(_stage_CD(b, st[b][2]))
                    if dbg and b == 0:
                        nc.gpsimd.dma_start(out=dbg_d["d_vts"], in_=st[0][0])
                        nc.gpsimd.dma_start(out=dbg_d["d_xsb"], in_=st[0][2])
                        nc.gpsimd.dma_start(out=dbg_d["d_xnT"], in_=dt[0][0])
                        nc.gpsimd.dma_start(out=dbg_d["d_q"], in_=dt[0][1])
                        nc.gpsimd.dma_start(out=dbg_d["d_kw"], in_=dt[0][2])
                if mark:
                    mark()
                for b in range(BPC):
                    _stage_FG(b, xs_ts[b], xf_ts[b], st[b][0], st[b][1],
                              dt[b][1], dt[b][2],
                              dbg_d if (dbg and b == 0) else None)
                    if mark and b < BPC - 1:
                        mark()

            if reps == 1:
                _samples_body()
            elif isinstance(reps, tuple):      # ("unroll", R)
                for _rep in range(reps[1]):
                    _samples_body()
            else:
                with tc.For_i(0, reps, 1, hint_engines=(mybir.EngineType.PE,),
                              staggered_reset=STAGGER):
                    _samples_body(tc.stage_boundary if STAGGER else None)

    nc.compile()
    return nc


def _prep(x_spa, x_freq, w_cdc, b_cdc, w_sv, b_sv, w_fv, b_fv,
          ln_w, ln_b, w_qk, w_spa, b_spa, w_frq, b_frq):
    """Host-side weight folding. Returns (flags, base input map)."""
    scale = float(HW) ** -0.5
    w_qk = np.asarray(w_qk, np.float32)
    ln_w = np.asarray(ln_w, np.float32)
    ln_b = np.asarray(ln_b, np.float32)
    w_spa = np.asarray(w_spa, np.float32)
    w_frq = np.asarray(w_frq, np.float32)

    wqkTg = w_qk.T * ln_w[:, None]            # [hw, 2hw]
    Wq = wqkTg[:, :HW]
    Wk = np.ascontiguousarray(wqkTg[:, HW:])
    Wspa = Wk @ (w_spa.T * scale)
    Wfrq = Wk @ (w_frq.T * scale)
    wbig = _bf(np.concatenate([Wq, Wspa, Wfrq], axis=1))

    qkb = ln_b @ w_qk.T                       # [2hw]
    qb = qkb[:HW]
    kb = qkb[HW:]
    dqb = np.concatenate([qb, kb @ (w_spa.T * scale), kb @ (w_frq.T * scale)])

    flags = (bool(np.any(dqb)), bool(np.any(b_spa)), bool(np.any(b_frq)),
             bool(np.any(b_sv)), bool(np.any(b_fv)))

    base = {
        "wcdcT": _round_f32r(np.asarray(w_cdc, np.float32).T),
        "wsvT": _round_f32r(np.asarray(w_sv, np.float32).T),
        "wfvT": _round_f32r(np.asarray(w_fv, np.float32).T),
        "wbig": wbig,
    }
    if flags[0]:
        base["dqb"] = _bf(dqb[None, :])
    if flags[1]:
        base["bspa"] = _bf(np.asarray(b_spa, np.float32)[None, :])
    if flags[2]:
        base["bfrq"] = _bf(np.asarray(b_frq, np.float32)[None, :])
    if flags[3]:
        base["bsv"] = _round_f32r(np.asarray(b_sv, np.float32)[None, :])
    if flags[4]:
        base["bfv"] = _round_f32r(np.asarray(b_fv, np.float32)[None, :])
    # b_cdc adds a per-row constant before row-LayerNorm: exact no-op.
    return flags, base


def kernel(x_spa, x_freq, w_cdc, b_cdc, w_sv, b_sv, w_fv, b_fv,
           ln_w, ln_b, w_qk, w_spa, b_spa, w_frq, b_frq):
    flags, base = _prep(x_spa, x_freq, w_cdc, b_cdc, w_sv, b_sv, w_fv, b_fv,
                        ln_w, ln_b, w_qk, w_spa, b_spa, w_frq, b_frq)

    if flags not in _CACHE:
        _CACHE[flags] = _build(flags)
    nc = _CACHE[flags]

    xs = _round_f32r(np.asarray(x_spa, np.float32).reshape(B, C, HW))
    xf = _round_f32r(np.asarray(x_freq, np.float32).reshape(B, C, HW))

    in_maps = []
    for c in range(NCORES):
        m = dict(base)
        m["xs"] = xs[c * BPC:(c + 1) * BPC]
        m["xf"] = xf[c * BPC:(c + 1) * BPC]
        in_maps.append(m)

    res = bass_utils.run_bass_kernel_spmd(nc, in_maps, core_ids=list(range(NCORES)))
    out_spa = np.concatenate([res.results[c]["os"] for c in range(NCORES)], axis=0)
    out_frq = np.concatenate([res.results[c]["of"] for c in range(NCORES)], axis=0)
    return (out_spa.reshape(B, C, H, W).astype(np.float32),
            out_frq.reshape(B, C, H, W).astype(np.float32))
